# revision 20
# baseline (speedup 1.0000x reference)
"""Multi-head attention on 8 Trainium2 NeuronCores (Bass/Tile).

Problem: x:[4096,512] -> q,k,v heads (H=8, d=64), per-head softmax(q k^T / 8) @ v,
output projection. Returns (output [4096,64], attention_weights [8,4096,4096]).

Sharding: one head per NeuronCore (8 heads / 8 cores). Each core receives the
full x (replicated) plus its head's weight slices, and computes:
  - X^T via PE transposes (hidden on partitions) for the projections
  - Q^T, K^T [64, 4096] and V_aug = [1 | V] [4096, 65] (ones col -> row sums)
  - scores transposed: S^T[k, q] = sum_d K[k,d] Q[q,d]  (f32r matmuls)
  - E^T = exp(S^T) unnormalized (ACT, scale 1/sqrt(d) folded into Q,K on host)
  - att_aug^T [65, 512] = sum_k V_aug[k,:]^T E^T[k,:]  (row 0 = Z = softmax denom)
  - zinv broadcast tile via K=1 outer-product matmul
  - A^T = E^T * zinv (DVE) -> DMA out; host un-transposes with a numpy view
  - P^T = woaug^T @ att_aug^T, normalized -> partial output per head
Host: sums per-head partial outputs, adds bias corrections (V bias folded
analytically: A rows sum to 1 => att = A@V_raw + bv).
"""

import math
import sys

import numpy as np

for _p in ("/opt/trn_rl_repo", "/opt/trn_rl_repo/concourse"):
    if _p not in sys.path:
        sys.path.insert(0, _p)

N_NODES = 4096
HIDDEN = 512
NUM_HEADS = 8
HEAD_DIM = 64
N_CORES = 8

P = 128                      # SBUF partitions
QCHUNK = 512                 # queries per main-loop chunk
D1 = HEAD_DIM + 1            # augmented dim (Z row first)

# set by test.py to collect timing; harness just calls kernel()
TRACE = False
LAST_RESULTS = None


def _emit(tc, aps, n_nodes=N_NODES):
    from contextlib import ExitStack

    import concourse.bass as bass
    import concourse.mybir as mybir

    nc = tc.nc
    ctx = ExitStack()
    f32 = mybir.dt.float32
    f32r = mybir.dt.float32r
    AF = mybir.ActivationFunctionType

    xt_d, wq_d, wk_d, wv_d, bq_d, bk_d, wo_d, attnT_d, outT_d = aps
    zscratch = nc.dram_tensor("zscratch", [n_nodes], mybir.dt.float32).ap()

    KB = n_nodes // P            # key blocks of 128
    KT = HIDDEN // P             # hidden k-tiles of 128
    NCH = n_nodes // QCHUNK      # main-loop chunks
    NQC = QCHUNK // P            # 128-row groups per chunk (for x tiling reuse)

    consts = ctx.enter_context(tc.tile_pool(name="consts", bufs=1))
    big = ctx.enter_context(tc.tile_pool(name="big", bufs=2))
    qkpool = ctx.enter_context(tc.tile_pool(name="qkpool", bufs=1))
    vpool = ctx.enter_context(tc.tile_pool(name="vpool", bufs=1))
    small = ctx.enter_context(tc.tile_pool(name="small", bufs=2))
    ps_s = ctx.enter_context(tc.tile_pool(name="ps_s", bufs=3, space="PSUM"))
    ps_f = ctx.enter_context(tc.tile_pool(name="ps_f", bufs=1, space="PSUM"))
    ps_av = ctx.enter_context(tc.tile_pool(name="ps_av", bufs=2, space="PSUM"))
    ps_m = ctx.enter_context(tc.tile_pool(name="ps_m", bufs=2, space="PSUM"))

    # ---- constants ----
    wq_t = consts.tile([P, KT, HEAD_DIM], f32r, tag="wq")
    wk_t = consts.tile([P, KT, HEAD_DIM], f32r, tag="wk")
    wv_t = consts.tile([P, KT, HEAD_DIM], f32r, tag="wv")
    nc.sync.dma_start(out=wq_t, in_=wq_d.rearrange("(kt p) d -> p kt d", p=P))
    nc.sync.dma_start(out=wk_t, in_=wk_d.rearrange("(kt p) d -> p kt d", p=P))
    nc.sync.dma_start(out=wv_t, in_=wv_d.rearrange("(kt p) d -> p kt d", p=P))
    bq_t = consts.tile([HEAD_DIM, 1], f32, tag="bq")
    bk_t = consts.tile([HEAD_DIM, 1], f32, tag="bk")
    nc.sync.dma_start(out=bq_t, in_=bq_d.rearrange("(p o) -> p o", o=1))
    nc.sync.dma_start(out=bk_t, in_=bk_d.rearrange("(p o) -> p o", o=1))
    woaug_t = consts.tile([D1, D1], f32, tag="woaug")
    nc.sync.dma_start(out=woaug_t, in_=wo_d)

    # ---- X^T: [hidden, n] as [128, KT, n] (host supplies x transposed) ----
    XT = big.tile([P, KT, n_nodes], f32r, tag="big")
    nc.sync.dma_start(out=XT, in_=xt_d.rearrange("(kt p) n -> p kt n", p=P))

    Vt = vpool.tile([P, KB, D1], f32r, tag="vaug")
    # fill with 1.0; V evacuations overwrite cols 1:D1, leaving the ones col
    nc.vector.memset(Vt[:, :, :].bitcast(f32), 1.0)

    # walrus allows only one sync-wait on f32/f32r matmuls, so absorb each
    # cross-engine producer's sem into a 1-element dummy matmul (a real PE
    # data dep -> exactly one wait each; later matmuls then see the
    # producer ticks as already-observed and emit no extra waits). One
    # accumulation group so the shared psum slot adds no same-engine waits.
    funnel_aps = [wq_t[:, 0, 0:1], wk_t[:, 0, 0:1], wv_t[:, 0, 0:1],
                  XT[:, 0, 0:1], woaug_t[:, 0:1], Vt[:, 0, 0:1]]
    pf = ps_f.tile([1, 1], f32, tag="pf")
    for i, ap in enumerate(funnel_aps):
        nc.tensor.matmul(pf, ap.bitcast(f32), ap.bitcast(f32),
                         start=(i == 0), stop=(i == len(funnel_aps) - 1))
    tc.no_sync_barrier()

    # ---- projections: Q^T, K^T [64, n] (scaled by 1/sqrt(8) host-side) ----
    QT = qkpool.tile([HEAD_DIM, n_nodes], f32r, tag="qt")
    KTt = qkpool.tile([HEAD_DIM, n_nodes], f32r, tag="kt")
    for ch in range(NCH):
        csl = slice(ch * QCHUNK, (ch + 1) * QCHUNK)
        for (wt, bt, dst) in ((wq_t, bq_t, QT), (wk_t, bk_t, KTt)):
            pq = ps_m.tile([HEAD_DIM, QCHUNK], f32, tag="pm")
            for kt in range(KT):
                nc.tensor.matmul(pq, wt[:, kt, :],
                                 XT[:, kt, csl],
                                 start=(kt == 0), stop=(kt == KT - 1))
            nc.scalar.activation(out=dst[:, csl], in_=pq, func=AF.Identity,
                                 bias=bt, scale=1.0)

    # ---- V_aug = [1 | V] as [128, KB, 65] ----
    for nb in range(KB):
        pv = ps_m.tile([P, HEAD_DIM], f32, tag="pm")
        for kt in range(KT):
            nc.tensor.matmul(pv, XT[:, kt, nb * P:(nb + 1) * P],
                             wv_t[:, kt, :],
                             start=(kt == 0), stop=(kt == KT - 1))
        nc.scalar.copy(out=Vt[:, nb, 1:D1], in_=pv)

    # ---- main loop over query chunks ----
    attnT_v = attnT_d.rearrange("(kb p) q -> p kb q", p=P)
    for ch in range(NCH):
        csl = slice(ch * QCHUNK, (ch + 1) * QCHUNK)
        ET = big.tile([P, KB, QCHUNK], f32r, tag="big")
        pav = ps_av.tile([D1, QCHUNK], f32, tag="pav")
        for kb in range(KB):
            ps = ps_s.tile([P, QCHUNK], f32, tag="ps")
            nc.tensor.matmul(ps, KTt[:, kb * P:(kb + 1) * P],
                             QT[:, csl])
            nc.scalar.activation(out=ET[:, kb, :], in_=ps, func=AF.Exp)
            nc.tensor.matmul(pav, Vt[:, kb, :],
                             ET[:, kb, :],
                             start=(kb == 0), stop=(kb == KB - 1))
        att_sb = small.tile([D1, QCHUNK], f32, tag="attsb")
        nc.scalar.copy(out=att_sb, in_=pav)
        # zinv = 1/Z (row 0 of att_sb)
        zr = small.tile([1, QCHUNK], f32, tag="zr")
        nc.vector.reciprocal(out=zr, in_=att_sb[0:1, :])
        # broadcast zinv to all 128 partitions via a tiny DRAM bounce
        zsl = zscratch[ch * QCHUNK:(ch + 1) * QCHUNK]
        nc.sync.dma_start(out=zsl.rearrange("(o q) -> o q", o=1), in_=zr)
        zb = small.tile([P, QCHUNK], f32, tag="zb")
        zbc = bass.AP(tensor=zsl.tensor, offset=zsl.offset,
                      ap=[[0, P], [1, QCHUNK]])
        nc.sync.dma_start(out=zb, in_=zbc)
        # normalize E^T in place: A^T = E^T * zinv[q]
        zb_ap = zb[:, :]
        zb_bcast = bass.AP(tensor=zb_ap.tensor, offset=zb_ap.offset,
                           ap=[zb_ap.ap[0], [0, KB], zb_ap.ap[1]])
        nc.vector.tensor_mul(ET, ET, zb_bcast)
        nc.sync.dma_start(out=attnT_v[:, :, csl], in_=ET[:, :, :].bitcast(f32))
        # output projection (+ Z row preserved), then normalize
        pp = ps_m.tile([D1, QCHUNK], f32, tag="pm")
        nc.tensor.matmul(pp, woaug_t, att_sb)
        ot = small.tile([D1, QCHUNK], f32, tag="ot")
        nc.scalar.copy(out=ot, in_=pp)
        nc.vector.tensor_mul(ot, ot, zb[0:D1, :])
        nc.sync.dma_start(out=outT_d[:, csl], in_=ot[1:D1, :])

    tc.strict_bb_all_engine_barrier()
    ctx.close()


def build_program(n_nodes=N_NODES):
    import concourse.mybir as mybir
    import concourse.tile as tile
    from concourse import bacc

    f32 = mybir.dt.float32
    f32r = mybir.dt.float32r
    nc = bacc.Bacc(trn_type="TRN2", target_bir_lowering=False, debug=False,
                   num_devices=N_CORES)
    aps = (
        nc.dram_tensor("xt", [HIDDEN, n_nodes], f32r, kind="ExternalInput").ap(),
        nc.dram_tensor("wq", [HIDDEN, HEAD_DIM], f32r, kind="ExternalInput").ap(),
        nc.dram_tensor("wk", [HIDDEN, HEAD_DIM], f32r, kind="ExternalInput").ap(),
        nc.dram_tensor("wv", [HIDDEN, HEAD_DIM], f32r, kind="ExternalInput").ap(),
        nc.dram_tensor("bq", [HEAD_DIM], f32, kind="ExternalInput").ap(),
        nc.dram_tensor("bk", [HEAD_DIM], f32, kind="ExternalInput").ap(),
        nc.dram_tensor("woaug", [D1, D1], f32, kind="ExternalInput").ap(),
        nc.dram_tensor("attnT", [n_nodes, n_nodes], f32, kind="ExternalOutput").ap(),
        nc.dram_tensor("outT", [HEAD_DIM, n_nodes], f32, kind="ExternalOutput").ap(),
    )
    with tile.TileContext(nc) as tc:
        _emit(tc, aps, n_nodes=n_nodes)
    nc.compile()
    return nc


_PROGRAM = None


def _get_program():
    global _PROGRAM
    if _PROGRAM is None:
        _PROGRAM = build_program()
    return _PROGRAM


def make_in_maps(x, Wq_w, Wq_b, Wk_w, Wk_b, Wv_w, Wv_b, Wo_w, Wo_b):
    a = np.float32(1.0 / math.sqrt(math.sqrt(float(HEAD_DIM))))  # d^-1/4
    xt = np.ascontiguousarray(np.asarray(x, dtype=np.float32).T)
    in_maps = []
    for h in range(NUM_HEADS):
        sl = slice(h * HEAD_DIM, (h + 1) * HEAD_DIM)
        woaug = np.zeros((D1, D1), dtype=np.float32)
        woaug[0, 0] = 1.0
        woaug[1:, 1:] = np.asarray(Wo_w)[sl, :]
        in_maps.append({
            "xt": xt,
            "wq": np.ascontiguousarray(np.asarray(Wq_w)[:, sl] * a, np.float32),
            "wk": np.ascontiguousarray(np.asarray(Wk_w)[:, sl] * a, np.float32),
            "wv": np.ascontiguousarray(np.asarray(Wv_w)[:, sl], np.float32),
            "bq": np.ascontiguousarray(np.asarray(Wq_b)[sl] * a, np.float32),
            "bk": np.ascontiguousarray(np.asarray(Wk_b)[sl] * a, np.float32),
            "woaug": woaug,
        })
    return in_maps


def assemble(results, Wv_b, Wo_w, Wo_b):
    """Gather per-core results into full outputs."""
    Wv_b = np.asarray(Wv_b, np.float32)
    Wo_w = np.asarray(Wo_w, np.float32)
    Wo_b = np.asarray(Wo_b, np.float32)
    attnT = np.empty((NUM_HEADS, N_NODES, N_NODES), dtype=np.float32)
    out = np.zeros((N_NODES, HEAD_DIM), dtype=np.float32)
    bias_corr = np.zeros((HEAD_DIM,), dtype=np.float32)
    for h in range(NUM_HEADS):
        r = results[h]
        attnT[h] = r["attnT"]
        out += r["outT"].T
        sl = slice(h * HEAD_DIM, (h + 1) * HEAD_DIM)
        bias_corr += Wv_b[sl] @ Wo_w[sl, :]
    out = out + bias_corr + Wo_b
    return out, attnT.transpose(0, 2, 1)


def _ensure_axon_profile_hook():
    """Install the NTFF profile hook that this image's antenv lacks.

    Only used for TRACE runs (timing/profiling in test.py); the plain
    kernel() path never touches it.
    """
    import types

    try:
        from antenv.axon_hooks import get_axon_ntff_profile_hook  # noqa: F401
        return
    except ImportError:
        pass
    holder = {"hook": None}
    mod = types.ModuleType("antenv.axon_hooks")
    mod.set_axon_ntff_profile_hook = lambda h: holder.__setitem__("hook", h)
    mod.get_axon_ntff_profile_hook = lambda: holder["hook"]
    sys.modules["antenv.axon_hooks"] = mod
    import antenv

    antenv.axon_hooks = mod
    try:
        from trn_agent_boot.trn_boot import _ntff_profile_via_ctypes

        hook = _ntff_profile_via_ctypes("/opt/axon/libaxon_pjrt.so")
        if hook is not None:
            mod.set_axon_ntff_profile_hook(hook)
    except Exception as e:  # degrade to no tracing
        print(f"profile hook unavailable: {e}", file=sys.stderr)
    # keep trace post-processing local (no artifact upload from sandbox)
    import concourse.bass_utils as bu

    bu.upload_artifacts = lambda tmpdir: tmpdir


def kernel(x, Wq_w, Wq_b, Wk_w, Wk_b, Wv_w, Wv_b, Wo_w, Wo_b):
    global LAST_RESULTS
    from concourse.bass_utils import run_bass_kernel_spmd

    if TRACE:
        _ensure_axon_profile_hook()
    nc = _get_program()
    in_maps = make_in_maps(x, Wq_w, Wq_b, Wk_w, Wk_b, Wv_w, Wv_b, Wo_w, Wo_b)
    res = run_bass_kernel_spmd(nc, in_maps, core_ids=list(range(N_CORES)),
                               trace=TRACE)
    LAST_RESULTS = res
    return assemble(res.results, Wv_b, Wo_w, Wo_b)


# revision 21
# speedup vs baseline: 1.0447x; 1.0447x over previous
"""Multi-head attention on 8 Trainium2 NeuronCores (Bass/Tile).

Problem: x:[4096,512] -> q,k,v heads (H=8, d=64), per-head softmax(q k^T / 8) @ v,
output projection. Returns (output [4096,64], attention_weights [8,4096,4096]).

Sharding: one head per NeuronCore (8 heads / 8 cores). Each core receives x^T
(replicated, host-transposed) plus its head's weight slices, and computes:
  - Q^T, K^T [128, 4096] (both partition halves hold the same head; the
    duplication lets score matmuls row-pack two K=64 contractions into the
    128x128 PE array concurrently via tile_position)
  - V_aug = [1 | V] [4096, 65] (ones col -> softmax denominators)
  - scores transposed: S^T[k, q] = sum_d K[k,d] Q[q,d]  (f32r matmuls,
    1/sqrt(d) folded into the host-prescaled Q,K weights)
  - E^T = exp(S^T) unnormalized (ACT, one op per psum bank pair)
  - att_aug^T [65, 512] = sum_k V_aug[k,:]^T E^T[k,:]  (row 0 = Z)
  - zinv broadcast to all partitions via a tiny DRAM bounce
  - A^T = E^T * zinv[q] (DVE) -> DMA out; host un-transposes with a view
  - P^T = woaug^T @ att_aug^T, normalized -> partial output per head
Host: sums per-head partial outputs, adds bias corrections (V bias folded
analytically: A rows sum to 1 => att = A@V_raw + bv).
"""

import math
import sys

import numpy as np

for _p in ("/opt/trn_rl_repo", "/opt/trn_rl_repo/concourse"):
    if _p not in sys.path:
        sys.path.insert(0, _p)

N_NODES = 4096
HIDDEN = 512
NUM_HEADS = 8
HEAD_DIM = 64
N_CORES = 8

P = 128                      # SBUF partitions
QCHUNK = 512                 # queries per main-loop chunk
D1 = HEAD_DIM + 1            # augmented dim (Z row first)

# set by test.py to collect timing; harness just calls kernel()
TRACE = False
LAST_RESULTS = None


def _emit(tc, aps, n_nodes=N_NODES):
    from contextlib import ExitStack

    import concourse.bass as bass
    import concourse.mybir as mybir

    nc = tc.nc
    ctx = ExitStack()
    f32 = mybir.dt.float32
    f32r = mybir.dt.float32r
    AF = mybir.ActivationFunctionType

    xt_d, wq_d, wk_d, wv_d, bq_d, bk_d, wo_d, attnT_d, outT_d = aps
    zscratch = nc.dram_tensor("zscratch", [n_nodes], mybir.dt.float32).ap()

    KB = n_nodes // P            # key blocks of 128
    KT = HIDDEN // P             # hidden k-tiles of 128
    NCH = n_nodes // QCHUNK      # main-loop chunks

    consts = ctx.enter_context(tc.tile_pool(name="consts", bufs=1))
    big = ctx.enter_context(tc.tile_pool(name="big", bufs=2))
    qkpool = ctx.enter_context(tc.tile_pool(name="qkpool", bufs=1))
    vpool = ctx.enter_context(tc.tile_pool(name="vpool", bufs=1))
    small = ctx.enter_context(tc.tile_pool(name="small", bufs=2))
    ps_s = ctx.enter_context(tc.tile_pool(name="ps_s", bufs=2, space="PSUM"))
    ps_av = ctx.enter_context(tc.tile_pool(name="ps_av", bufs=2, space="PSUM"))
    ps_m = ctx.enter_context(tc.tile_pool(name="ps_m", bufs=2, space="PSUM"))

    # ---- constants (wq/wk host-duplicated to width 128) ----
    wq_t = consts.tile([P, KT, P], f32r, tag="wq")
    wk_t = consts.tile([P, KT, P], f32r, tag="wk")
    wv_t = consts.tile([P, KT, HEAD_DIM], f32r, tag="wv")
    nc.sync.dma_start(out=wq_t, in_=wq_d.rearrange("(kt p) d -> p kt d", p=P))
    nc.sync.dma_start(out=wk_t, in_=wk_d.rearrange("(kt p) d -> p kt d", p=P))
    nc.sync.dma_start(out=wv_t, in_=wv_d.rearrange("(kt p) d -> p kt d", p=P))
    bq_t = consts.tile([P, 1], f32, tag="bq")
    bk_t = consts.tile([P, 1], f32, tag="bk")
    nc.sync.dma_start(out=bq_t, in_=bq_d.rearrange("(p o) -> p o", o=1))
    nc.sync.dma_start(out=bk_t, in_=bk_d.rearrange("(p o) -> p o", o=1))
    woaug_t = consts.tile([D1, D1], f32, tag="woaug")
    nc.sync.dma_start(out=woaug_t, in_=wo_d)

    # ---- X^T: [hidden, n] as [128, KT, n] (host supplies x transposed) ----
    XT = big.tile([P, KT, n_nodes], f32r, tag="big")
    nc.sync.dma_start(out=XT, in_=xt_d.rearrange("(kt p) n -> p kt n", p=P))

    Vt = vpool.tile([P, KB, D1], f32r, tag="vaug")
    # fill with 1.0; V evacuations overwrite cols 1:D1, leaving the ones col
    nc.vector.memset(Vt[:, :, :].bitcast(f32), 1.0)

    # ---- projections: Q^T, K^T duplicated on both partition halves ----
    QT = qkpool.tile([P, n_nodes], f32r, tag="qt")
    KTt = qkpool.tile([P, n_nodes], f32r, tag="kt")
    for ch in range(NCH):
        csl = slice(ch * QCHUNK, (ch + 1) * QCHUNK)
        for (wt, bt, dst) in ((wq_t, bq_t, QT), (wk_t, bk_t, KTt)):
            pq = ps_m.tile([P, QCHUNK], f32, tag="pm")
            for kt in range(KT):
                nc.tensor.matmul(pq, wt[:, kt, :], XT[:, kt, csl],
                                 start=(kt == 0), stop=(kt == KT - 1))
            nc.scalar.activation(out=dst[:, csl], in_=pq, func=AF.Identity,
                                 bias=bt, scale=1.0)

    # ---- V_aug = [1 | V] as [128, KB, 65] ----
    for nb in range(KB):
        pv = ps_m.tile([P, HEAD_DIM], f32, tag="pm")
        for kt in range(KT):
            nc.tensor.matmul(pv, XT[:, kt, nb * P:(nb + 1) * P],
                             wv_t[:, kt, :],
                             start=(kt == 0), stop=(kt == KT - 1))
        nc.scalar.copy(out=Vt[:, nb, 1:D1], in_=pv)

    # ---- main loop over query chunks ----
    attnT_v = attnT_d.rearrange("(kb p) q -> p kb q", p=P)
    for ch in range(NCH):
        csl = slice(ch * QCHUNK, (ch + 1) * QCHUNK)
        ET = big.tile([P, KB, QCHUNK], f32r, tag="big")
        pav = ps_av.tile([D1, QCHUNK], f32, tag="pav")
        for kb2 in range(KB // 2):
            kbA, kbB = 2 * kb2, 2 * kb2 + 1
            ps2 = ps_s.tile([P, 2 * QCHUNK], f32, tag="ps")
            # two K=64 score matmuls packed into the array's row halves
            nc.tensor.matmul(ps2[:, 0:QCHUNK],
                             KTt[0:HEAD_DIM, kbA * P:(kbA + 1) * P],
                             QT[0:HEAD_DIM, csl])
            nc.tensor.matmul(ps2[:, QCHUNK:2 * QCHUNK],
                             KTt[HEAD_DIM:P, kbB * P:(kbB + 1) * P],
                             QT[HEAD_DIM:P, csl])
            nc.scalar.activation(out=ET[:, kbA:kbA + 2, :], in_=ps2,
                                 func=AF.Exp)
            nc.tensor.matmul(pav, Vt[:, kbA, :], ET[:, kbA, :],
                             start=(kb2 == 0), stop=False)
            nc.tensor.matmul(pav, Vt[:, kbB, :], ET[:, kbB, :],
                             start=False, stop=(kb2 == KB // 2 - 1))
        att_sb = small.tile([D1, QCHUNK], f32, tag="attsb")
        nc.scalar.copy(out=att_sb, in_=pav)
        # zinv = 1/Z (row 0 of att_sb)
        zr = small.tile([1, QCHUNK], f32, tag="zr")
        nc.vector.reciprocal(out=zr, in_=att_sb[0:1, :])
        # broadcast zinv to all 128 partitions via a tiny DRAM bounce
        zsl = zscratch[ch * QCHUNK:(ch + 1) * QCHUNK]
        nc.sync.dma_start(out=zsl.rearrange("(o q) -> o q", o=1), in_=zr)
        zb = small.tile([P, QCHUNK], f32, tag="zb")
        zbc = bass.AP(tensor=zsl.tensor, offset=zsl.offset,
                      ap=[[0, P], [1, QCHUNK]])
        nc.sync.dma_start(out=zb, in_=zbc)
        # normalize E^T in place: A^T = E^T * zinv[q]
        zb_ap = zb[:, :]
        zb_bcast = bass.AP(tensor=zb_ap.tensor, offset=zb_ap.offset,
                           ap=[zb_ap.ap[0], [0, KB], zb_ap.ap[1]])
        nc.vector.tensor_mul(ET, ET, zb_bcast)
        nc.sync.dma_start(out=attnT_v[:, :, csl], in_=ET[:, :, :].bitcast(f32))
        # output projection (+ Z row preserved), then normalize
        pp = ps_m.tile([D1, QCHUNK], f32, tag="pm")
        nc.tensor.matmul(pp, woaug_t, att_sb)
        ot = small.tile([D1, QCHUNK], f32, tag="ot")
        nc.scalar.copy(out=ot, in_=pp)
        nc.vector.tensor_mul(ot, ot, zb[0:D1, :])
        nc.sync.dma_start(out=outT_d[:, csl], in_=ot[1:D1, :])

    ctx.close()


def build_program(n_nodes=N_NODES):
    import concourse.mybir as mybir
    import concourse.tile as tile
    from concourse import bacc

    f32 = mybir.dt.float32
    f32r = mybir.dt.float32r
    nc = bacc.Bacc(trn_type="TRN2", target_bir_lowering=False, debug=False,
                   num_devices=N_CORES)
    aps = (
        nc.dram_tensor("xt", [HIDDEN, n_nodes], f32r, kind="ExternalInput").ap(),
        nc.dram_tensor("wq", [HIDDEN, P], f32r, kind="ExternalInput").ap(),
        nc.dram_tensor("wk", [HIDDEN, P], f32r, kind="ExternalInput").ap(),
        nc.dram_tensor("wv", [HIDDEN, HEAD_DIM], f32r, kind="ExternalInput").ap(),
        nc.dram_tensor("bq", [P], f32, kind="ExternalInput").ap(),
        nc.dram_tensor("bk", [P], f32, kind="ExternalInput").ap(),
        nc.dram_tensor("woaug", [D1, D1], f32, kind="ExternalInput").ap(),
        nc.dram_tensor("attnT", [n_nodes, n_nodes], f32, kind="ExternalOutput").ap(),
        nc.dram_tensor("outT", [HEAD_DIM, n_nodes], f32, kind="ExternalOutput").ap(),
    )
    with tile.TileContext(nc) as tc:
        _emit(tc, aps, n_nodes=n_nodes)
    nc.compile()
    return nc


_PROGRAM = None


def _get_program():
    global _PROGRAM
    if _PROGRAM is None:
        _PROGRAM = build_program()
    return _PROGRAM


def make_in_maps(x, Wq_w, Wq_b, Wk_w, Wk_b, Wv_w, Wv_b, Wo_w, Wo_b):
    a = np.float32(1.0 / math.sqrt(math.sqrt(float(HEAD_DIM))))  # d^-1/4
    xt = np.ascontiguousarray(np.asarray(x, dtype=np.float32).T)
    in_maps = []
    for h in range(NUM_HEADS):
        sl = slice(h * HEAD_DIM, (h + 1) * HEAD_DIM)
        woaug = np.zeros((D1, D1), dtype=np.float32)
        woaug[0, 0] = 1.0
        woaug[1:, 1:] = np.asarray(Wo_w)[sl, :]
        wq = np.asarray(Wq_w)[:, sl].astype(np.float32) * a
        wk = np.asarray(Wk_w)[:, sl].astype(np.float32) * a
        bq = np.asarray(Wq_b)[sl].astype(np.float32) * a
        bk = np.asarray(Wk_b)[sl].astype(np.float32) * a
        in_maps.append({
            "xt": xt,
            "wq": np.ascontiguousarray(np.concatenate([wq, wq], axis=1)),
            "wk": np.ascontiguousarray(np.concatenate([wk, wk], axis=1)),
            "wv": np.ascontiguousarray(np.asarray(Wv_w)[:, sl], np.float32),
            "bq": np.ascontiguousarray(np.concatenate([bq, bq])),
            "bk": np.ascontiguousarray(np.concatenate([bk, bk])),
            "woaug": woaug,
        })
    return in_maps


def assemble(results, Wv_b, Wo_w, Wo_b):
    """Gather per-core results into full outputs."""
    Wv_b = np.asarray(Wv_b, np.float32)
    Wo_w = np.asarray(Wo_w, np.float32)
    Wo_b = np.asarray(Wo_b, np.float32)
    attnT = np.empty((NUM_HEADS, N_NODES, N_NODES), dtype=np.float32)
    out = np.zeros((N_NODES, HEAD_DIM), dtype=np.float32)
    bias_corr = np.zeros((HEAD_DIM,), dtype=np.float32)
    for h in range(NUM_HEADS):
        r = results[h]
        attnT[h] = r["attnT"]
        out += r["outT"].T
        sl = slice(h * HEAD_DIM, (h + 1) * HEAD_DIM)
        bias_corr += Wv_b[sl] @ Wo_w[sl, :]
    out = out + bias_corr + Wo_b
    return out, attnT.transpose(0, 2, 1)


def _ensure_axon_profile_hook():
    """Install the NTFF profile hook that this image's antenv lacks.

    Only used for TRACE runs (timing/profiling in test.py); the plain
    kernel() path never touches it.
    """
    import types

    try:
        from antenv.axon_hooks import get_axon_ntff_profile_hook  # noqa: F401
        return
    except ImportError:
        pass
    holder = {"hook": None}
    mod = types.ModuleType("antenv.axon_hooks")
    mod.set_axon_ntff_profile_hook = lambda h: holder.__setitem__("hook", h)
    mod.get_axon_ntff_profile_hook = lambda: holder["hook"]
    sys.modules["antenv.axon_hooks"] = mod
    import antenv

    antenv.axon_hooks = mod
    try:
        from trn_agent_boot.trn_boot import _ntff_profile_via_ctypes

        hook = _ntff_profile_via_ctypes("/opt/axon/libaxon_pjrt.so")
        if hook is not None:
            mod.set_axon_ntff_profile_hook(hook)
    except Exception as e:  # degrade to no tracing
        print(f"profile hook unavailable: {e}", file=sys.stderr)
    # keep trace post-processing local (no artifact upload from sandbox)
    import concourse.bass_utils as bu

    bu.upload_artifacts = lambda tmpdir: tmpdir


def kernel(x, Wq_w, Wq_b, Wk_w, Wk_b, Wv_w, Wv_b, Wo_w, Wo_b):
    global LAST_RESULTS
    from concourse.bass_utils import run_bass_kernel_spmd

    if TRACE:
        _ensure_axon_profile_hook()
    nc = _get_program()
    in_maps = make_in_maps(x, Wq_w, Wq_b, Wk_w, Wk_b, Wv_w, Wv_b, Wo_w, Wo_b)
    res = run_bass_kernel_spmd(nc, in_maps, core_ids=list(range(N_CORES)),
                               trace=TRACE)
    LAST_RESULTS = res
    return assemble(res.results, Wv_b, Wo_w, Wo_b)


# revision 23
# speedup vs baseline: 1.4257x; 1.3647x over previous
"""Multi-head attention on 8 Trainium2 NeuronCores (Bass/Tile).

Problem: x:[4096,512] -> q,k,v heads (H=8, d=64), per-head softmax(q k^T / 8) @ v,
output projection. Returns (output [4096,64], attention_weights [8,4096,4096]).

Sharding: one head per NeuronCore (8 heads / 8 cores). Each core receives x^T
(replicated, host-transposed) plus its head's weight slices, and computes:
  - Q^T, K^T [128, 4096] (both partition halves hold the same head; the
    duplication lets score matmuls row-pack two K=64 contractions into the
    128x128 PE array concurrently via tile_position)
  - V_aug = [1 | V] [4096, 65] (ones col -> softmax denominators)
  - scores transposed: S^T[k, q] = sum_d K[k,d] Q[q,d]  (f32r matmuls,
    1/sqrt(d) folded into the host-prescaled Q,K weights)
  - E^T = exp(S^T) unnormalized (ACT, one op per psum bank pair)
  - att_aug^T [65, 512] = sum_k V_aug[k,:]^T E^T[k,:]  (row 0 = Z)
  - zinv broadcast to all partitions via a tiny DRAM bounce
  - A^T = E^T * zinv[q] (DVE) -> DMA out; host un-transposes with a view
  - P^T = woaug^T @ att_aug^T, normalized -> partial output per head
Host: sums per-head partial outputs, adds bias corrections (V bias folded
analytically: A rows sum to 1 => att = A@V_raw + bv).
"""

import math
import sys

import numpy as np

for _p in ("/opt/trn_rl_repo", "/opt/trn_rl_repo/concourse"):
    if _p not in sys.path:
        sys.path.insert(0, _p)

N_NODES = 4096
HIDDEN = 512
NUM_HEADS = 8
HEAD_DIM = 64
N_CORES = 8

P = 128                      # SBUF partitions
QCHUNK = 512                 # queries per main-loop chunk
D1 = HEAD_DIM + 1            # augmented dim (Z row first)

# set by test.py to collect timing; harness just calls kernel()
TRACE = False
LAST_RESULTS = None


def _emit(tc, aps, n_nodes=N_NODES):
    from contextlib import ExitStack

    import concourse.bass as bass
    import concourse.mybir as mybir

    nc = tc.nc
    ctx = ExitStack()
    f32 = mybir.dt.float32
    f32r = mybir.dt.float32r
    AF = mybir.ActivationFunctionType

    xt_d, wq_d, wk_d, wv_d, bq_d, bk_d, wo_d, attnT_d, outT_d = aps
    zscratch = nc.dram_tensor("zscratch", [n_nodes], mybir.dt.float32).ap()

    KB = n_nodes // P            # key blocks of 128
    KT = HIDDEN // P             # hidden k-tiles of 128
    NCH = n_nodes // QCHUNK      # main-loop chunks

    consts = ctx.enter_context(tc.tile_pool(name="consts", bufs=1))
    big = ctx.enter_context(tc.tile_pool(name="big", bufs=2))
    qkpool = ctx.enter_context(tc.tile_pool(name="qkpool", bufs=1))
    vpool = ctx.enter_context(tc.tile_pool(name="vpool", bufs=1))
    small = ctx.enter_context(tc.tile_pool(name="small", bufs=2))
    ps_s = ctx.enter_context(tc.tile_pool(name="ps_s", bufs=2, space="PSUM"))
    ps_av = ctx.enter_context(tc.tile_pool(name="ps_av", bufs=2, space="PSUM"))
    ps_m = ctx.enter_context(tc.tile_pool(name="ps_m", bufs=2, space="PSUM"))

    # ---- constants (wq/wk host-duplicated to width 128) ----
    wq_t = consts.tile([P, KT, P], f32r, tag="wq")
    wk_t = consts.tile([P, KT, P], f32r, tag="wk")
    wv_t = consts.tile([P, KT, HEAD_DIM], f32r, tag="wv")
    nc.sync.dma_start(out=wq_t, in_=wq_d.rearrange("(kt p) d -> p kt d", p=P))
    nc.sync.dma_start(out=wk_t, in_=wk_d.rearrange("(kt p) d -> p kt d", p=P))
    nc.sync.dma_start(out=wv_t, in_=wv_d.rearrange("(kt p) d -> p kt d", p=P))
    bq_t = consts.tile([P, 1], f32, tag="bq")
    bk_t = consts.tile([P, 1], f32, tag="bk")
    nc.sync.dma_start(out=bq_t, in_=bq_d.rearrange("(p o) -> p o", o=1))
    nc.sync.dma_start(out=bk_t, in_=bk_d.rearrange("(p o) -> p o", o=1))
    woaug_t = consts.tile([D1, D1], f32, tag="woaug")
    nc.sync.dma_start(out=woaug_t, in_=wo_d)

    # ---- X^T: [hidden, n] as [128, KT, n] (host supplies x transposed) ----
    XT = big.tile([P, KT, n_nodes], f32r, tag="big")
    nc.sync.dma_start(out=XT, in_=xt_d.rearrange("(kt p) n -> p kt n", p=P))

    Vt = vpool.tile([P, KB, D1], f32r, tag="vaug")
    # fill with 1.0; V evacuations overwrite cols 1:D1, leaving the ones col
    nc.vector.memset(Vt[:, :, :].bitcast(f32), 1.0)

    # ---- projections: Q^T, K^T duplicated on both partition halves ----
    QT = qkpool.tile([P, n_nodes], f32r, tag="qt")
    KTt = qkpool.tile([P, n_nodes], f32r, tag="kt")
    for ch in range(NCH):
        csl = slice(ch * QCHUNK, (ch + 1) * QCHUNK)
        for (wt, bt, dst) in ((wq_t, bq_t, QT), (wk_t, bk_t, KTt)):
            pq = ps_m.tile([P, QCHUNK], f32, tag="pm")
            for kt in range(KT):
                nc.tensor.matmul(pq, wt[:, kt, :], XT[:, kt, csl],
                                 start=(kt == 0), stop=(kt == KT - 1))
            nc.scalar.activation(out=dst[:, csl], in_=pq, func=AF.Identity,
                                 bias=bt, scale=1.0)

    # ---- V_aug = [1 | V] as [128, KB, 65] ----
    for nb in range(KB):
        pv = ps_m.tile([P, HEAD_DIM], f32, tag="pm")
        for kt in range(KT):
            nc.tensor.matmul(pv, XT[:, kt, nb * P:(nb + 1) * P],
                             wv_t[:, kt, :],
                             start=(kt == 0), stop=(kt == KT - 1))
        nc.scalar.copy(out=Vt[:, nb, 1:D1], in_=pv)

    # ---- main loop over query chunks ----
    attnT_v = attnT_d.rearrange("(kb p) q -> p kb q", p=P)
    for ch in range(NCH):
        csl = slice(ch * QCHUNK, (ch + 1) * QCHUNK)
        ET = big.tile([P, KB, QCHUNK], f32r, tag="big")
        pav = ps_av.tile([D1, QCHUNK], f32, tag="pav")
        for kb2 in range(KB // 2):
            kbA, kbB = 2 * kb2, 2 * kb2 + 1
            ps2 = ps_s.tile([P, 2 * QCHUNK], f32, tag="ps")
            # two K=64 score matmuls packed into the array's row halves
            nc.tensor.matmul(ps2[:, 0:QCHUNK],
                             KTt[0:HEAD_DIM, kbA * P:(kbA + 1) * P],
                             QT[0:HEAD_DIM, csl])
            nc.tensor.matmul(ps2[:, QCHUNK:2 * QCHUNK],
                             KTt[HEAD_DIM:P, kbB * P:(kbB + 1) * P],
                             QT[HEAD_DIM:P, csl])
            nc.scalar.activation(out=ET[:, kbA:kbA + 2, :], in_=ps2,
                                 func=AF.Exp)
            nc.tensor.matmul(pav, Vt[:, kbA, :], ET[:, kbA, :],
                             start=(kb2 == 0), stop=False)
            nc.tensor.matmul(pav, Vt[:, kbB, :], ET[:, kbB, :],
                             start=False, stop=(kb2 == KB // 2 - 1))
        att_sb = small.tile([D1, QCHUNK], f32, tag="attsb")
        nc.scalar.copy(out=att_sb, in_=pav)
        # zinv = exp(-ln Z) on ACT (DVE reciprocal is 8 cyc/elem iterative;
        # exp+log share one ACT table set)
        zr = small.tile([1, QCHUNK], f32, tag="zr")
        nc.scalar.activation(out=zr, in_=att_sb[0:1, :], func=AF.Ln)
        nc.scalar.activation(out=zr, in_=zr, func=AF.Exp, scale=-1.0)
        # broadcast zinv to all 128 partitions via a tiny DRAM bounce
        zsl = zscratch[ch * QCHUNK:(ch + 1) * QCHUNK]
        nc.sync.dma_start(out=zsl.rearrange("(o q) -> o q", o=1), in_=zr)
        zb = small.tile([P, QCHUNK], f32, tag="zb")
        zbc = bass.AP(tensor=zsl.tensor, offset=zsl.offset,
                      ap=[[0, P], [1, QCHUNK]])
        nc.sync.dma_start(out=zb, in_=zbc)
        # normalize E^T in place (A^T = E^T * zinv[q]) and stream out in
        # kb-pair pieces so DVE work and the attnT DMA overlap
        zb_ap = zb[:, :]
        zb_bcast = bass.AP(tensor=zb_ap.tensor, offset=zb_ap.offset,
                           ap=[zb_ap.ap[0], [0, 2], zb_ap.ap[1]])
        for kb2 in range(KB // 2):
            kbA = 2 * kb2
            sl2 = ET[:, kbA:kbA + 2, :]
            nc.vector.tensor_mul(sl2, sl2, zb_bcast)
            nc.sync.dma_start(out=attnT_v[:, kbA:kbA + 2, csl],
                              in_=sl2.bitcast(f32))
        # output projection (+ Z row preserved), then normalize
        pp = ps_m.tile([D1, QCHUNK], f32, tag="pm")
        nc.tensor.matmul(pp, woaug_t, att_sb)
        ot = small.tile([D1, QCHUNK], f32, tag="ot")
        nc.scalar.copy(out=ot, in_=pp)
        nc.vector.tensor_mul(ot, ot, zb[0:D1, :])
        nc.sync.dma_start(out=outT_d[:, csl], in_=ot[1:D1, :])

    ctx.close()


def build_program(n_nodes=N_NODES):
    import concourse.mybir as mybir
    import concourse.tile as tile
    from concourse import bacc

    f32 = mybir.dt.float32
    f32r = mybir.dt.float32r
    nc = bacc.Bacc(trn_type="TRN2", target_bir_lowering=False, debug=False,
                   num_devices=N_CORES)
    aps = (
        nc.dram_tensor("xt", [HIDDEN, n_nodes], f32r, kind="ExternalInput").ap(),
        nc.dram_tensor("wq", [HIDDEN, P], f32r, kind="ExternalInput").ap(),
        nc.dram_tensor("wk", [HIDDEN, P], f32r, kind="ExternalInput").ap(),
        nc.dram_tensor("wv", [HIDDEN, HEAD_DIM], f32r, kind="ExternalInput").ap(),
        nc.dram_tensor("bq", [P], f32, kind="ExternalInput").ap(),
        nc.dram_tensor("bk", [P], f32, kind="ExternalInput").ap(),
        nc.dram_tensor("woaug", [D1, D1], f32, kind="ExternalInput").ap(),
        nc.dram_tensor("attnT", [n_nodes, n_nodes], f32, kind="ExternalOutput").ap(),
        nc.dram_tensor("outT", [HEAD_DIM, n_nodes], f32, kind="ExternalOutput").ap(),
    )
    with tile.TileContext(nc) as tc:
        _emit(tc, aps, n_nodes=n_nodes)
    nc.compile()
    return nc


_PROGRAM = None


def _get_program():
    global _PROGRAM
    if _PROGRAM is None:
        _PROGRAM = build_program()
    return _PROGRAM


def make_in_maps(x, Wq_w, Wq_b, Wk_w, Wk_b, Wv_w, Wv_b, Wo_w, Wo_b):
    a = np.float32(1.0 / math.sqrt(math.sqrt(float(HEAD_DIM))))  # d^-1/4
    xt = np.ascontiguousarray(np.asarray(x, dtype=np.float32).T)
    in_maps = []
    for h in range(NUM_HEADS):
        sl = slice(h * HEAD_DIM, (h + 1) * HEAD_DIM)
        woaug = np.zeros((D1, D1), dtype=np.float32)
        woaug[0, 0] = 1.0
        woaug[1:, 1:] = np.asarray(Wo_w)[sl, :]
        wq = np.asarray(Wq_w)[:, sl].astype(np.float32) * a
        wk = np.asarray(Wk_w)[:, sl].astype(np.float32) * a
        bq = np.asarray(Wq_b)[sl].astype(np.float32) * a
        bk = np.asarray(Wk_b)[sl].astype(np.float32) * a
        in_maps.append({
            "xt": xt,
            "wq": np.ascontiguousarray(np.concatenate([wq, wq], axis=1)),
            "wk": np.ascontiguousarray(np.concatenate([wk, wk], axis=1)),
            "wv": np.ascontiguousarray(np.asarray(Wv_w)[:, sl], np.float32),
            "bq": np.ascontiguousarray(np.concatenate([bq, bq])),
            "bk": np.ascontiguousarray(np.concatenate([bk, bk])),
            "woaug": woaug,
        })
    return in_maps


def assemble(results, Wv_b, Wo_w, Wo_b):
    """Gather per-core results into full outputs."""
    Wv_b = np.asarray(Wv_b, np.float32)
    Wo_w = np.asarray(Wo_w, np.float32)
    Wo_b = np.asarray(Wo_b, np.float32)
    attnT = np.empty((NUM_HEADS, N_NODES, N_NODES), dtype=np.float32)
    out = np.zeros((N_NODES, HEAD_DIM), dtype=np.float32)
    bias_corr = np.zeros((HEAD_DIM,), dtype=np.float32)
    for h in range(NUM_HEADS):
        r = results[h]
        attnT[h] = r["attnT"]
        out += r["outT"].T
        sl = slice(h * HEAD_DIM, (h + 1) * HEAD_DIM)
        bias_corr += Wv_b[sl] @ Wo_w[sl, :]
    out = out + bias_corr + Wo_b
    return out, attnT.transpose(0, 2, 1)


def _ensure_axon_profile_hook():
    """Install the NTFF profile hook that this image's antenv lacks.

    Only used for TRACE runs (timing/profiling in test.py); the plain
    kernel() path never touches it.
    """
    import types

    try:
        from antenv.axon_hooks import get_axon_ntff_profile_hook  # noqa: F401
        return
    except ImportError:
        pass
    holder = {"hook": None}
    mod = types.ModuleType("antenv.axon_hooks")
    mod.set_axon_ntff_profile_hook = lambda h: holder.__setitem__("hook", h)
    mod.get_axon_ntff_profile_hook = lambda: holder["hook"]
    sys.modules["antenv.axon_hooks"] = mod
    import antenv

    antenv.axon_hooks = mod
    try:
        from trn_agent_boot.trn_boot import _ntff_profile_via_ctypes

        hook = _ntff_profile_via_ctypes("/opt/axon/libaxon_pjrt.so")
        if hook is not None:
            mod.set_axon_ntff_profile_hook(hook)
    except Exception as e:  # degrade to no tracing
        print(f"profile hook unavailable: {e}", file=sys.stderr)
    # keep trace post-processing local (no artifact upload from sandbox)
    import concourse.bass_utils as bu

    bu.upload_artifacts = lambda tmpdir: tmpdir


def kernel(x, Wq_w, Wq_b, Wk_w, Wk_b, Wv_w, Wv_b, Wo_w, Wo_b):
    global LAST_RESULTS
    from concourse.bass_utils import run_bass_kernel_spmd

    if TRACE:
        _ensure_axon_profile_hook()
    nc = _get_program()
    in_maps = make_in_maps(x, Wq_w, Wq_b, Wk_w, Wk_b, Wv_w, Wv_b, Wo_w, Wo_b)
    res = run_bass_kernel_spmd(nc, in_maps, core_ids=list(range(N_CORES)),
                               trace=TRACE)
    LAST_RESULTS = res
    return assemble(res.results, Wv_b, Wo_w, Wo_b)
